# revision 1
# baseline (speedup 1.0000x reference)
"""GAT kernel builder + host prep for nn_GATOnlyNet on 8 trn2 cores.

Algorithm (SPMD, edges sorted by dst, disjoint per-core dst-node ranges):
  per layer:
    Phase 1: z_ext[v, 0:136] = h[v] @ Wext for ALL nodes (redundant per core),
      Wext = [W.T | W.T@Msrc | W.T@Mdst]; written to local DRAM table (fp32).
    Phase 2: per dst-tile (128 nodes) / 512-edge group / 128-edge block:
      gather z_ext rows by src (indirect DMA, 1 desc/edge);
      onehot[e,n] + onehotT[n,e] built on-chip (iota + PE ones-broadcast);
      s_dst per edge via PE matmul from the tile's s_dst rows;
      exp_e = exp(lrelu(s_src+s_dst)) (no max-subtraction needed: |e| <~ 15);
      aggT[hc,n] += msg.T@onehot, denT[4,n] += exp.T@onehot in PSUM;
      finalize: ELU(aggT / (denT+1e-9)) -> hT shard (bf16) -> AllGather,
      or (layer 3) logits via head_w.
"""
import numpy as np
from contextlib import ExitStack

import concourse.bass as bass
import concourse.tile as tile
from concourse import bacc, mybir
from concourse.bass import IndirectOffsetOnAxis

P = 128
IN_DIM = 128
HEADS = 4
COUT = 32
HC = HEADS * COUT           # 128
DZ = HC + 8                 # 136
NEG = 0.2
NLAYERS = 3
GROUP = 512
BPG = GROUP // P


def make_cfg(V, ncores, tiles_per_core):
    VSH = tiles_per_core * P
    return dict(V=V, Vp=ncores * VSH, ncores=ncores, VSH=VSH, TILES=tiles_per_core)


def host_prep(cfg, x, edge_index, Ws, a_src, a_dst, head_w, head_b):
    V, Vp, NC, VSH, TILES = cfg["V"], cfg["Vp"], cfg["ncores"], cfg["VSH"], cfg["TILES"]
    src = np.asarray(edge_index[0], np.int64)
    dst = np.asarray(edge_index[1], np.int64)
    order = np.argsort(dst, kind="stable")
    src, dst = src[order], dst[order]

    core_of = (dst // VSH).astype(np.int64)
    tile_of = ((dst % VSH) // P).astype(np.int64)

    counts = np.zeros((NC, TILES), np.int64)
    for c in range(NC):
        counts[c] = np.bincount(tile_of[core_of == c], minlength=TILES)
    b_per_slot = np.maximum(1, -(-counts.max(axis=0) // P)).astype(np.int64)
    g_per_slot = -(-b_per_slot // BPG)
    NB = int(b_per_slot.sum())
    NG = int(g_per_slot.sum())
    blk0 = np.concatenate([[0], np.cumsum(b_per_slot)])[:-1].astype(np.int64)
    grp0 = np.concatenate([[0], np.cumsum(g_per_slot)])[:-1].astype(np.int64)

    src_cols = np.zeros((NC, P, NB), np.int32)
    dstl_col = np.full((NC, P, NB), -1.0, np.float32)
    dstl_row = np.full((NC, NG, GROUP), -1.0, np.float32)
    tnodes = np.zeros((NC, P, TILES), np.int32)

    for c in range(NC):
        m = core_of == c
        s_c, d_c, t_c = src[m], dst[m], tile_of[m]
        for k in range(TILES):
            tnodes[c, :, k] = c * VSH + k * P + np.arange(P)
            mk = t_c == k
            sk = s_c[mk].astype(np.int32)
            dk = (d_c[mk] - (c * VSH + k * P)).astype(np.float32)
            nslots = int(b_per_slot[k]) * P
            ngslots = int(g_per_slot[k]) * GROUP
            sk_p = np.zeros(ngslots, np.int32)
            dk_p = np.full(ngslots, -1.0, np.float32)
            sk_p[:len(sk)] = sk
            dk_p[:len(dk)] = dk
            for g in range(int(g_per_slot[k])):
                dstl_row[c, int(grp0[k]) + g, :] = dk_p[g * GROUP:(g + 1) * GROUP]
            for b in range(int(b_per_slot[k])):
                col = int(blk0[k]) + b
                src_cols[c, :, col] = sk_p[b * P:(b + 1) * P]
                dstl_col[c, :, col] = dk_p[b * P:(b + 1) * P]

    Wext = np.zeros((NLAYERS, IN_DIM, DZ), np.float32)
    for li in range(NLAYERS):
        W = np.asarray(Ws[li], np.float32)
        Msl = np.zeros((HC, HEADS), np.float32)
        Mdl = np.zeros((HC, HEADS), np.float32)
        for h in range(HEADS):
            Msl[h * COUT:(h + 1) * COUT, h] = np.asarray(a_src[li])[h]
            Mdl[h * COUT:(h + 1) * COUT, h] = np.asarray(a_dst[li])[h]
        Wext[li, :, 0:HC] = W.T
        Wext[li, :, HC:HC + 4] = W.T @ Msl
        Wext[li, :, HC + 4:HC + 8] = W.T @ Mdl

    xT = np.zeros((IN_DIM, Vp), np.float32)
    xT[:, :V] = np.asarray(x, np.float32).T
    E4 = np.zeros((4, P), np.float32)
    for h in range(HEADS):
        E4[h, h * COUT:(h + 1) * COUT] = 1.0
    hw = np.asarray(head_w, np.float32).reshape(HC, 1)
    hb = float(np.asarray(head_b).reshape(-1)[0])

    meta = dict(NG=NG, NB=NB, g_per_slot=[int(v) for v in g_per_slot],
                b_per_slot=[int(v) for v in b_per_slot],
                blk0=[int(v) for v in blk0], grp0=[int(v) for v in grp0], hb=hb)
    in_maps = []
    for c in range(NC):
        in_maps.append({
            "xT": xT, "Wext": Wext, "E4": E4, "head_w": hw,
            "src_cols": src_cols[c], "dstl_col": dstl_col[c],
            "dstl_row": dstl_row[c], "tnodes": tnodes[c],
        })
    return in_maps, meta


def build_nc(cfg, meta, repeat=1):
    Vp, NC, VSH, TILES = cfg["Vp"], cfg["ncores"], cfg["VSH"], cfg["TILES"]
    NG, NB = meta["NG"], meta["NB"]
    g_per_slot, blk0, grp0, hb = meta["g_per_slot"], meta["blk0"], meta["grp0"], meta["hb"]
    b_per_slot = meta["b_per_slot"]
    NCH = Vp // P

    nc = bacc.Bacc("TRN2", target_bir_lowering=False, debug=False, num_devices=NC)
    f32, bf16, i32 = mybir.dt.float32, mybir.dt.bfloat16, mybir.dt.int32

    xT = nc.dram_tensor("xT", [IN_DIM, Vp], f32, kind="ExternalInput").ap()
    Wext = nc.dram_tensor("Wext", [NLAYERS, IN_DIM, DZ], f32, kind="ExternalInput").ap()
    E4d = nc.dram_tensor("E4", [4, P], f32, kind="ExternalInput").ap()
    hwd = nc.dram_tensor("head_w", [HC, 1], f32, kind="ExternalInput").ap()
    srcd = nc.dram_tensor("src_cols", [P, NB], i32, kind="ExternalInput").ap()
    dcold = nc.dram_tensor("dstl_col", [P, NB], f32, kind="ExternalInput").ap()
    drowd = nc.dram_tensor("dstl_row", [NG, GROUP], f32, kind="ExternalInput").ap()
    tnd = nc.dram_tensor("tnodes", [P, TILES], i32, kind="ExternalInput").ap()
    out = nc.dram_tensor("out", [VSH], f32, kind="ExternalOutput").ap()

    ztab = nc.dram_tensor("ztab", [Vp, DZ], f32)
    hsh = nc.dram_tensor("hsh", [IN_DIM, VSH], bf16)
    hfull = nc.dram_tensor("hfull", [NC, IN_DIM, VSH], bf16, addr_space="Shared")

    with tile.TileContext(nc) as tc, ExitStack() as ctx:
        cst = ctx.enter_context(tc.tile_pool(name="cst", bufs=1))
        sbz = ctx.enter_context(tc.tile_pool(name="sbz", bufs=16))
        sbg = ctx.enter_context(tc.tile_pool(name="sbg", bufs=6))
        sbt = ctx.enter_context(tc.tile_pool(name="sbt", bufs=2))
        sb1 = ctx.enter_context(tc.tile_pool(name="sb1", bufs=4))
        psS = ctx.enter_context(tc.tile_pool(name="psS", bufs=2, space="PSUM"))
        psB = ctx.enter_context(tc.tile_pool(name="psB", bufs=2, space="PSUM"))
        psD = ctx.enter_context(tc.tile_pool(name="psD", bufs=2, space="PSUM"))
        psA = ctx.enter_context(tc.tile_pool(name="psA", bufs=2, space="PSUM"))

        srci = cst.tile([P, NB], i32)
        nc.sync.dma_start(out=srci[:], in_=srcd[:, :])
        dcol = cst.tile([P, NB], f32)
        nc.sync.dma_start(out=dcol[:], in_=dcold[:, :])
        tns = cst.tile([P, TILES], i32)
        nc.sync.dma_start(out=tns[:], in_=tnd[:, :])
        WextS = cst.tile([IN_DIM, NLAYERS * DZ], f32)
        for li in range(NLAYERS):
            nc.sync.dma_start(out=WextS[:, li * DZ:(li + 1) * DZ], in_=Wext[li, :, :])
        E4S = cst.tile([4, P], f32)
        nc.sync.dma_start(out=E4S[:], in_=E4d[:, :])
        hwS = cst.tile([HC, 1], f32)
        nc.sync.dma_start(out=hwS[:], in_=hwd[:, :])
        onesS = cst.tile([1, P], f32)
        nc.vector.memset(onesS[:], 1.0)
        iota_i = cst.tile([P, P], i32)
        nc.gpsimd.iota(iota_i[:], pattern=[[1, P]], base=0, channel_multiplier=0)
        iotaR = cst.tile([P, P], f32)
        nc.vector.tensor_copy(out=iotaR[:], in_=iota_i[:])
        iota_p = cst.tile([P, 1], i32)
        nc.gpsimd.iota(iota_p[:], pattern=[[0, 1]], base=0, channel_multiplier=1)
        iotaP = cst.tile([P, 1], f32)
        nc.vector.tensor_copy(out=iotaP[:], in_=iota_p[:])
        logitS = cst.tile([1, VSH], f32)

        def elu(xn):
            tmin = sbt.tile([P, P], f32, tag="tmin")
            nc.vector.tensor_scalar(out=tmin[:], in0=xn[:], scalar1=0.0, scalar2=None,
                                    op0=mybir.AluOpType.min)
            texp = sbt.tile([P, P], f32, tag="texp")
            nc.scalar.activation(out=texp[:], in_=tmin[:],
                                 func=mybir.ActivationFunctionType.Exp)
            trel = sbt.tile([P, P], f32, tag="trel")
            nc.vector.tensor_scalar(out=trel[:], in0=xn[:], scalar1=0.0, scalar2=None,
                                    op0=mybir.AluOpType.max)
            hsum = sbt.tile([P, P], f32, tag="hsum")
            nc.vector.tensor_add(out=hsum[:], in0=trel[:], in1=texp[:])
            return hsum  # still needs -1

        for _rep in range(repeat):
         for li in range(NLAYERS):
            WextL = WextS[:, li * DZ:(li + 1) * DZ]
            # ---------- Phase 1 (512-node wide chunks) ----------
            WCH = 4 * P
            for wc in range(Vp // WCH):
                if li == 0:
                    hT = sb1.tile([IN_DIM, WCH], f32, tag="hT")
                    nc.sync.dma_start(out=hT[:], in_=xT[:, wc * WCH:(wc + 1) * WCH])
                else:
                    hTb = sb1.tile([IN_DIM, WCH], bf16, tag="hTb")
                    # hfull is [NC, 128, VSH]; a 512-node run may cross shard blocks
                    n0 = wc * WCH
                    done = 0
                    while done < WCH:
                        g0 = n0 + done
                        blkc, colc = g0 // VSH, g0 % VSH
                        w = min(WCH - done, VSH - colc)
                        nc.sync.dma_start(out=hTb[:, done:done + w],
                                          in_=hfull.ap()[blkc, :, colc:colc + w])
                        done += w
                    hT = sb1.tile([IN_DIM, WCH], f32, tag="hT")
                    nc.vector.tensor_copy(out=hT[:], in_=hTb[:])
                for sub in range(4):
                    ch = wc * 4 + sub
                    zc = psS.tile([P, DZ], f32, space="PSUM", tag="small")
                    nc.tensor.matmul(out=zc[:], lhsT=hT[:, sub * P:(sub + 1) * P],
                                     rhs=WextL, start=True, stop=True)
                    zs = sb1.tile([P, DZ], f32, tag="zs")
                    nc.vector.tensor_copy(out=zs[:], in_=zc[:])
                    nc.scalar.dma_start(out=ztab.ap()[ch * P:(ch + 1) * P, :], in_=zs[:])

            # ---------- Phase 2 ----------
            for k in range(TILES):
                aggT = psA.tile([P, P], f32, space="PSUM", tag="aggT")
                denT = psD.tile([4, P], f32, space="PSUM", tag="denT")
                sdstS = sbt.tile([P, 4], f32, tag="sdstS")
                nc.gpsimd.indirect_dma_start(
                    out=sdstS[:], out_offset=None, in_=ztab.ap()[:, :],
                    in_offset=IndirectOffsetOnAxis(ap=tns[:, k:k + 1], axis=0),
                    element_offset=HC + 4)
                for g in range(g_per_slot[k]):
                    gi = grp0[k] + g
                    nbl = min(BPG, b_per_slot[k] - g * BPG)
                    drow = sbg.tile([1, GROUP], f32, tag="drow")
                    nc.sync.dma_start(out=drow[:], in_=drowd[gi, :][None, :])
                    dbc = psB.tile([P, GROUP], f32, space="PSUM", tag="dbc")
                    nc.tensor.matmul(out=dbc[:], lhsT=onesS[:], rhs=drow[:],
                                     start=True, stop=True)
                    ohT = sbg.tile([P, GROUP], f32, tag="ohT")
                    nc.vector.tensor_scalar(out=ohT[:], in0=dbc[:], scalar1=iotaP[:, :1],
                                            scalar2=None, op0=mybir.AluOpType.is_equal)
                    for b in range(nbl):
                        col = blk0[k] + g * BPG + b
                        first = (g == 0 and b == 0)
                        last = (g == g_per_slot[k] - 1 and b == nbl - 1)
                        ge = sbz.tile([P, DZ], f32, tag="ge")
                        nc.gpsimd.indirect_dma_start(
                            out=ge[:], out_offset=None, in_=ztab.ap()[:, :],
                            in_offset=IndirectOffsetOnAxis(ap=srci[:, col:col + 1], axis=0))
                        oh = sbg.tile([P, P], bf16, tag="oh")
                        nc.vector.tensor_scalar(out=oh[:], in0=iotaR[:],
                                                scalar1=dcol[:, col:col + 1],
                                                scalar2=None, op0=mybir.AluOpType.is_equal)
                        sde = psS.tile([P, 4], f32, space="PSUM", tag="small")
                        nc.tensor.matmul(out=sde[:], lhsT=ohT[:, b * P:(b + 1) * P],
                                         rhs=sdstS[:], start=True, stop=True)
                        esc = sbg.tile([P, 4], f32, tag="esc")
                        nc.vector.tensor_add(out=esc[:], in0=ge[:, HC:HC + 4], in1=sde[:])
                        esc2 = sbg.tile([P, 4], f32, tag="esc2")
                        nc.vector.tensor_scalar(out=esc2[:], in0=esc[:], scalar1=NEG,
                                                scalar2=None, op0=mybir.AluOpType.mult)
                        nc.vector.tensor_tensor(out=esc[:], in0=esc[:], in1=esc2[:],
                                                op=mybir.AluOpType.max)
                        expf = sbg.tile([P, 4], f32, tag="expf")
                        nc.scalar.activation(out=expf[:], in_=esc[:],
                                             func=mybir.ActivationFunctionType.Exp)
                        expb = sbg.tile([P, 4], bf16, tag="expb")
                        nc.vector.tensor_copy(out=expb[:], in_=expf[:])
                        msg = sbg.tile([P, HC], bf16, tag="msg")
                        for h in range(HEADS):
                            nc.vector.tensor_scalar(
                                out=msg[:, h * COUT:(h + 1) * COUT],
                                in0=ge[:, h * COUT:(h + 1) * COUT],
                                scalar1=expf[:, h:h + 1], scalar2=None,
                                op0=mybir.AluOpType.mult)
                        nc.tensor.matmul(out=aggT[:], lhsT=msg[:], rhs=oh[:],
                                         start=first, stop=last, skip_group_check=True)
                        nc.tensor.matmul(out=denT[:], lhsT=expb[:], rhs=oh[:],
                                         start=first, stop=last, skip_group_check=True)
                # ---- finalize ----
                dsb = sbt.tile([4, P], f32, tag="dsb")
                nc.vector.tensor_scalar(out=dsb[:], in0=denT[:], scalar1=1e-9,
                                        scalar2=None, op0=mybir.AluOpType.add)
                nc.vector.reciprocal(out=dsb[:], in_=dsb[:])
                rex = psS.tile([P, P], f32, space="PSUM", tag="small")
                nc.tensor.matmul(out=rex[:], lhsT=E4S[:], rhs=dsb[:], start=True, stop=True)
                rexS = sbt.tile([P, P], f32, tag="rexS")
                nc.vector.tensor_copy(out=rexS[:], in_=rex[:])
                xn = sbt.tile([P, P], f32, tag="xn")
                nc.vector.tensor_tensor(out=xn[:], in0=aggT[:], in1=rexS[:],
                                        op=mybir.AluOpType.mult)
                hsum = elu(xn)
                if li < NLAYERS - 1:
                    hb16 = sbt.tile([P, P], bf16, tag="hb16")
                    nc.vector.tensor_scalar(out=hb16[:], in0=hsum[:], scalar1=-1.0,
                                            scalar2=None, op0=mybir.AluOpType.add)
                    nc.scalar.dma_start(out=hsh.ap()[:, k * P:(k + 1) * P], in_=hb16[:])
                else:
                    h3 = sbt.tile([P, P], f32, tag="h3")
                    nc.vector.tensor_scalar(out=h3[:], in0=hsum[:], scalar1=-1.0,
                                            scalar2=None, op0=mybir.AluOpType.add)
                    lg = psS.tile([1, P], f32, space="PSUM", tag="small")
                    nc.tensor.matmul(out=lg[:], lhsT=hwS[:], rhs=h3[:], start=True,
                                     stop=True)
                    nc.vector.tensor_scalar(out=logitS[:, k * P:(k + 1) * P], in0=lg[:],
                                            scalar1=hb, scalar2=None,
                                            op0=mybir.AluOpType.add)
            if li < NLAYERS - 1:
                nc.gpsimd.collective_compute(
                    "AllGather", mybir.AluOpType.bypass,
                    replica_groups=[list(range(NC))],
                    ins=[hsh.ap()[:, :]], outs=[hfull.ap()[:, :, :]])
        nc.sync.dma_start(out=out[None, :], in_=logitS[:])
    nc.compile()
    return nc


def gat_reference_np(x, edge_index, Ws, a_src, a_dst, head_w, head_b):
    """Numpy reference (same math as reference.py) for small-scale validation."""
    V = x.shape[0]
    src = np.asarray(edge_index[0]); dst = np.asarray(edge_index[1])
    h = np.asarray(x, np.float64)
    for li in range(len(Ws)):
        z = (h @ np.asarray(Ws[li], np.float64).T).reshape(V, HEADS, COUT)
        ss = np.einsum("vhc,hc->vh", z, np.asarray(a_src[li], np.float64))
        sd = np.einsum("vhc,hc->vh", z, np.asarray(a_dst[li], np.float64))
        e = ss[src] + sd[dst]
        e = np.where(e > 0, e, NEG * e)
        m = np.full((V, HEADS), -np.inf); np.maximum.at(m, dst, e)
        m = np.maximum(m, -1e9)
        ex = np.exp(e - m[dst])
        den = np.zeros((V, HEADS)); np.add.at(den, dst, ex)
        alpha = ex / (den[dst] + 1e-9)
        msg = z[src] * alpha[:, :, None]
        agg = np.zeros((V, HEADS, COUT)); np.add.at(agg, dst, msg)
        h = np.where(agg > 0, agg, np.expm1(agg)).reshape(V, HC)
    return (h @ np.asarray(head_w, np.float64).T + np.asarray(head_b)).reshape(V)


# ======================= runner =======================

import time
import numpy as np
import jax
from jax.sharding import Mesh, PartitionSpec
from jax.experimental.shard_map import shard_map

import concourse.mybir as mybir
from concourse import bass2jax
from concourse.bass2jax import _bass_exec_p, install_neuronx_cc_hook, partition_id_tensor


class SpmdRunner:
    def __init__(self, nc, n_cores: int):
        install_neuronx_cc_hook()
        assert nc.dbg_addr is None or not nc.dbg_callbacks
        self.nc = nc
        self.n_cores = n_cores
        partition_name = nc.partition_id_tensor.name if nc.partition_id_tensor else None

        in_names, out_names, out_avals, zero_outs = [], [], [], []
        for alloc in nc.m.functions[0].allocations:
            if not isinstance(alloc, mybir.MemoryLocationSet):
                continue
            name = alloc.memorylocations[0].name
            if alloc.kind == "ExternalInput":
                if name != partition_name and name != (nc.dbg_addr.name if nc.dbg_addr else None):
                    in_names.append(name)
            elif alloc.kind == "ExternalOutput":
                out_names.append(name)
                shape = tuple(alloc.tensor_shape)
                dtype = mybir.dt.np(alloc.dtype)
                out_avals.append(jax.core.ShapedArray(shape, dtype))
                zero_outs.append(np.zeros(shape, dtype))
        self.in_names, self.out_names = in_names, out_names
        self.out_avals, self.zero_outs = out_avals, zero_outs
        n_params = len(in_names)
        self.n_params = n_params
        n_outs = len(out_avals)

        all_in_names = list(in_names) + list(out_names)
        if nc.dbg_addr is not None:
            all_in_names.append(nc.dbg_addr.name)
        if partition_name is not None:
            all_in_names.append(partition_name)

        dbg_name = nc.dbg_addr.name if nc.dbg_addr is not None else None

        def _body(*args):
            operands = list(args)
            if dbg_name is not None:
                operands.append(np.zeros((1, 2), np.uint32))
            if partition_name is not None:
                operands.append(partition_id_tensor())
            outs = _bass_exec_p.bind(
                *operands,
                out_avals=tuple(out_avals),
                in_names=tuple(all_in_names),
                out_names=tuple(out_names),
                lowering_input_output_aliases=(),
                sim_require_finite=True,
                sim_require_nnan=True,
                nc=nc,
            )
            return tuple(outs)

        devices = jax.devices()[:n_cores]
        assert len(devices) == n_cores
        self.mesh = Mesh(np.asarray(devices), ("core",))
        in_specs = (PartitionSpec("core"),) * (n_params + n_outs)
        out_specs = (PartitionSpec("core"),) * n_outs
        self.donate = tuple(range(n_params, n_params + n_outs))
        self.fn = jax.jit(
            shard_map(_body, mesh=self.mesh, in_specs=in_specs,
                      out_specs=out_specs, check_rep=False),
            donate_argnums=self.donate, keep_unused=True,
        )
        self.concat_in = None

    def load_inputs(self, in_maps):
        """Concat per-core inputs and push to devices once."""
        assert len(in_maps) == self.n_cores
        per_core = [[np.asarray(m[name]) for name in self.in_names] for m in in_maps]
        concat = [np.concatenate([per_core[c][i] for c in range(self.n_cores)], axis=0)
                  for i in range(self.n_params)]
        sh = jax.sharding.NamedSharding(self.mesh, PartitionSpec("core"))
        self.concat_in = [jax.device_put(a, sh) for a in concat]

    def _zeros(self):
        sh = jax.sharding.NamedSharding(self.mesh, PartitionSpec("core"))
        return [jax.device_put(np.zeros((self.n_cores * z.shape[0], *z.shape[1:]), z.dtype), sh)
                for z in self.zero_outs]

    def run(self):
        outs = self.fn(*self.concat_in, *self._zeros())
        jax.block_until_ready(outs)
        return [
            {name: np.asarray(outs[i]).reshape(self.n_cores, *self.out_avals[i].shape)[c]
             for i, name in enumerate(self.out_names)}
            for c in range(self.n_cores)
        ]

    def time(self, iters=8, warmup=2):
        """Per-call wall time (s) for the jitted executable, zeros pre-staged."""
        zs = [self._zeros() for _ in range(iters + warmup)]
        for i in range(warmup):
            jax.block_until_ready(self.fn(*self.concat_in, *zs[i]))
        ts = []
        for i in range(iters):
            t0 = time.perf_counter()
            jax.block_until_ready(self.fn(*self.concat_in, *zs[warmup + i]))
            ts.append(time.perf_counter() - t0)
        return min(ts), ts


# ======================= driver (self-contained kernel) =======================
import jax as _jax

_CACHE = {}
LAST_EXEC_NS = None


def _floor_nc(ncores):
    """Tiny kernel to estimate the per-call dispatch floor."""
    nc = bacc.Bacc("TRN2", target_bir_lowering=False, debug=False, num_devices=ncores)
    a = nc.dram_tensor("a", [P, 64], mybir.dt.float32, kind="ExternalInput").ap()
    b = nc.dram_tensor("b", [P, 64], mybir.dt.float32, kind="ExternalOutput").ap()
    with tile.TileContext(nc) as tc, ExitStack() as ctx:
        sb = ctx.enter_context(tc.tile_pool(name="sb", bufs=2))
        t = sb.tile([P, 64], mybir.dt.float32)
        nc.sync.dma_start(out=t[:], in_=a[:, :])
        nc.sync.dma_start(out=b[:, :], in_=t[:])
    nc.compile()
    return nc


def kernel(x, edge_index, Ws, a_src, a_dst, head_w, head_b):
    NC = 8
    V = int(np.asarray(x).shape[0])
    cfg = make_cfg(V, NC, tiles_per_core=(V + NC * P - 1) // (NC * P))
    in_maps, meta = host_prep(cfg, x, edge_index, Ws, a_src, a_dst, head_w, head_b)
    key = (V, tuple(meta["g_per_slot"]), tuple(meta["b_per_slot"]))
    if key not in _CACHE:
        nc = build_nc(cfg, meta)
        r = SpmdRunner(nc, NC)
        _CACHE[key] = r
    r = _CACHE[key]
    r.load_inputs(in_maps)
    res = r.run()
    out = np.concatenate([res[c]["out"] for c in range(NC)])[:V]
    return out.astype(np.float32)


def measure(iters=16):
    """Estimate HW exec ns via interleaved kernel/floor timing (drift-robust)."""
    import time as _time
    global LAST_EXEC_NS
    assert _CACHE, "call kernel() first"
    r = next(iter(_CACHE.values()))
    fnc = _floor_nc(r.n_cores)
    fr = SpmdRunner(fnc, r.n_cores)
    fr.load_inputs([{"a": np.zeros((P, 64), np.float32)}] * r.n_cores)
    fr.run()
    r.run()
    diffs, ks, fs = [], [], []
    for _ in range(iters):
        z = r._zeros()
        t0 = _time.perf_counter()
        _jax.block_until_ready(r.fn(*r.concat_in, *z))
        tk = _time.perf_counter() - t0
        zf = fr._zeros()
        t0 = _time.perf_counter()
        _jax.block_until_ready(fr.fn(*fr.concat_in, *zf))
        tf = _time.perf_counter() - t0
        ks.append(tk); fs.append(tf); diffs.append(tk - tf)
    diffs.sort()
    med = diffs[len(diffs) // 2]
    LAST_EXEC_NS = int(max(0.0, med) * 1e9)
    return LAST_EXEC_NS, sorted(ks)[len(ks)//2], sorted(fs)[len(fs)//2]



# revision 11
# speedup vs baseline: 1.1764x; 1.1764x over previous
"""GAT kernel for nn_GATOnlyNet on 8 trn2 cores (v2, group-batched).

Algorithm (SPMD, edges sorted by dst, disjoint per-core dst-node ranges):
  per layer:
    Phase 1: z_ext[v] = h[v] @ Wext for ALL nodes (redundant per core),
      Wext = [W.T | W.T@Msrc | W.T@Mdst]; stored in a DRAM table with
      packed rows: [128 x bf16 z | 4 x f32 s_src | 4 x f32 s_dst] = 288B.
    Phase 2: per dst-tile (128 nodes) / 512-edge group:
      ONE batched indirect gather (512 descriptors) of z rows by src;
      ohT[n,e] built via PE broadcast matmul + ACT abs/relu chain;
      s_dst per edge via 4 small PE matmuls; esc/lrelu/exp batched on
      [128,16]; msg = z * exp via one broadcast tensor_tensor;
      aggN[n,hc] / denN[n,h] accumulate in PSUM via per-block matmuls;
      finalize: ELU(aggN * 1/denN) -> transpose -> hsh shard (bf16) ->
      AllGather, or (layer 3) logits via head_w.
"""
import numpy as np
from contextlib import ExitStack

import concourse.bass as bass
import concourse.tile as tile
from concourse import bacc, mybir
from concourse.bass import IndirectOffsetOnAxis

P = 128
IN_DIM = 128
HEADS = 4
COUT = 32
HC = HEADS * COUT           # 128
DZ = HC + 8                 # 136 f32 values from the z matmul
DZU = 256                   # u16 units per packed ztab row (512B, 256B-aligned)
NEG = 0.2
NLAYERS = 3
GROUP = 512
BPG = GROUP // P            # 4
IPC = GROUP // 16           # idx cols per group (wrapped in 16 partitions)


def make_cfg(V, ncores, tiles_per_core):
    VSH = tiles_per_core * P
    return dict(V=V, Vp=ncores * VSH, ncores=ncores, VSH=VSH, TILES=tiles_per_core)


def _to_bf16(x):
    import ml_dtypes
    return np.asarray(x, np.float32).astype(ml_dtypes.bfloat16)


def host_prep(cfg, x, edge_index, Ws, a_src, a_dst, head_w, head_b):
    V, Vp, NC, VSH, TILES = cfg["V"], cfg["Vp"], cfg["ncores"], cfg["VSH"], cfg["TILES"]
    src = np.asarray(edge_index[0], np.int64)
    dst = np.asarray(edge_index[1], np.int64)
    order = np.argsort(dst, kind="stable")
    src, dst = src[order], dst[order]

    core_of = (dst // VSH).astype(np.int64)
    tile_of = ((dst % VSH) // P).astype(np.int64)

    HALF = Vp // 2
    cntA = np.zeros((NC, TILES), np.int64)
    cntB = np.zeros((NC, TILES), np.int64)
    for c in range(NC):
        m = core_of == c
        t_c, s_c = tile_of[m], src[m]
        a = s_c < HALF
        cntA[c] = np.bincount(t_c[a], minlength=TILES)
        cntB[c] = np.bincount(t_c[~a], minlength=TILES)
    gA = (-(-cntA.max(axis=0) // GROUP)).astype(np.int64)
    gB = (-(-cntB.max(axis=0) // GROUP)).astype(np.int64)
    gA = np.maximum(gA, 1)
    g_per_slot = gA + gB
    b_per_slot = g_per_slot * BPG
    NB = int(b_per_slot.sum())
    NG = int(g_per_slot.sum())
    blk0 = np.concatenate([[0], np.cumsum(b_per_slot)])[:-1].astype(np.int64)
    grp0 = np.concatenate([[0], np.cumsum(g_per_slot)])[:-1].astype(np.int64)

    idx16 = np.zeros((NC, 16, NG * IPC), np.int16)
    dstl_col = np.full((NC, P, NB), -1.0, np.float32)
    drow2 = np.zeros((NC, 2, NG * GROUP), np.float32)
    drow2[:, 0, :] = -1.0
    drow2[:, 1, :] = 1.0
    tnodes = np.zeros((NC, P, TILES), np.int32)

    for c in range(NC):
        m = core_of == c
        s_c, d_c, t_c = src[m], dst[m], tile_of[m]
        for k in range(TILES):
            tnodes[c, :, k] = c * VSH + k * P + np.arange(P)
            mk = t_c == k
            sk = s_c[mk].astype(np.int64)
            dk = (d_c[mk] - (c * VSH + k * P)).astype(np.float32)
            a = sk < HALF
            nA, nB_ = int(gA[k]) * GROUP, int(gB[k]) * GROUP
            sk_p = np.zeros(nA + nB_, np.int64)
            dk_p = np.full(nA + nB_, -1.0, np.float32)
            sk_p[:a.sum()] = sk[a]
            dk_p[:a.sum()] = dk[a]
            sk_p[nA:nA + (~a).sum()] = sk[~a] - HALF
            dk_p[nA:nA + (~a).sum()] = dk[~a]
            for b in range(int(b_per_slot[k])):
                col = int(blk0[k]) + b
                dstl_col[c, :, col] = dk_p[b * P:(b + 1) * P]
            g0 = int(grp0[k])
            drow2[c, 0, g0 * GROUP:(g0 + int(g_per_slot[k])) * GROUP] = dk_p
            for g in range(int(g_per_slot[k])):
                gi = g0 + g
                seg = sk_p[g * GROUP:(g + 1) * GROUP]
                wrapped = seg.reshape(IPC, 16).T.astype(np.int16)
                idx16[c, :, gi * IPC:(gi + 1) * IPC] = wrapped

    Wext = np.zeros((NLAYERS, IN_DIM, DZ), np.float32)
    for li in range(NLAYERS):
        W = np.asarray(Ws[li], np.float32)
        Msl = np.zeros((HC, HEADS), np.float32)
        Mdl = np.zeros((HC, HEADS), np.float32)
        for h in range(HEADS):
            Msl[h * COUT:(h + 1) * COUT, h] = np.asarray(a_src[li])[h]
            Mdl[h * COUT:(h + 1) * COUT, h] = np.asarray(a_dst[li])[h]
        Wext[li, :, 0:HC] = W.T
        Wext[li, :, HC:HC + 4] = W.T @ Msl
        Wext[li, :, HC + 4:HC + 8] = W.T @ Mdl

    xT = np.zeros((IN_DIM, Vp), np.float32)
    xT[:, :V] = np.asarray(x, np.float32).T
    lhsT2 = np.ones((2, P), np.float32)
    lhsT2[1] = -np.arange(P)
    iden = np.eye(P, dtype=np.float32)
    hw = np.asarray(head_w, np.float32).reshape(HC, 1)
    hb = float(np.asarray(head_b).reshape(-1)[0])

    meta = dict(NG=NG, NB=NB, g_per_slot=[int(v) for v in g_per_slot],
                gA=[int(v) for v in gA], gB=[int(v) for v in gB],
                b_per_slot=[int(v) for v in b_per_slot],
                blk0=[int(v) for v in blk0], grp0=[int(v) for v in grp0], hb=hb)
    in_maps = []
    for c in range(NC):
        in_maps.append({
            "xT": _to_bf16(xT), "Wext": _to_bf16(Wext),
            "lhsT2": _to_bf16(lhsT2), "iden": _to_bf16(iden),
            "head_w": _to_bf16(hw),
            "idx16": np.tile(idx16[c], (8, 1)), "dcol": dstl_col[c],
            "drow2": _to_bf16(drow2[c]), "tnodes": tnodes[c],
        })
    return in_maps, meta


def build_nc(cfg, meta, repeat=1):
    Vp, NC, VSH, TILES = cfg["Vp"], cfg["ncores"], cfg["VSH"], cfg["TILES"]
    NG, NB = meta["NG"], meta["NB"]
    g_per_slot, blk0, grp0, hb = meta["g_per_slot"], meta["blk0"], meta["grp0"], meta["hb"]
    gA, gB = meta["gA"], meta["gB"]
    GMAX = max(g_per_slot)
    HALF = Vp // 2

    nc = bacc.Bacc("TRN2", target_bir_lowering=False, debug=False, num_devices=NC)
    f32, bf16, i32 = mybir.dt.float32, mybir.dt.bfloat16, mybir.dt.int32
    u16 = mybir.dt.uint16
    AF = mybir.ActivationFunctionType
    OP = mybir.AluOpType

    xT = nc.dram_tensor("xT", [IN_DIM, Vp], bf16, kind="ExternalInput").ap()
    Wext = nc.dram_tensor("Wext", [NLAYERS, IN_DIM, DZ], bf16, kind="ExternalInput").ap()
    l2d = nc.dram_tensor("lhsT2", [2, P], bf16, kind="ExternalInput").ap()
    idend = nc.dram_tensor("iden", [P, P], bf16, kind="ExternalInput").ap()
    hwd = nc.dram_tensor("head_w", [HC, 1], bf16, kind="ExternalInput").ap()
    i16 = mybir.dt.int16
    idxd = nc.dram_tensor("idx16", [P, NG * IPC], i16, kind="ExternalInput").ap()
    dcold = nc.dram_tensor("dcol", [P, NB], f32, kind="ExternalInput").ap()
    drow2d = nc.dram_tensor("drow2", [2, NG * GROUP], bf16, kind="ExternalInput").ap()
    tnd = nc.dram_tensor("tnodes", [P, TILES], i32, kind="ExternalInput").ap()
    out = nc.dram_tensor("out", [VSH], f32, kind="ExternalOutput").ap()

    ztab = nc.dram_tensor("ztab", [Vp, DZU], u16)
    hsh = nc.dram_tensor("hsh", [IN_DIM, VSH], bf16)
    hfull = nc.dram_tensor("hfull", [NC, IN_DIM, VSH], bf16, addr_space="Shared")

    with tile.TileContext(nc) as tc, ExitStack() as ctx:
        cst = ctx.enter_context(tc.tile_pool(name="cst", bufs=1))
        sb1 = ctx.enter_context(tc.tile_pool(name="sb1", bufs=3))
        sbzs = ctx.enter_context(tc.tile_pool(name="sbzs", bufs=4))
        sbsd = ctx.enter_context(tc.tile_pool(name="sbsd", bufs=2))
        sbr2 = ctx.enter_context(tc.tile_pool(name="sbr2", bufs=2))
        sbz = ctx.enter_context(tc.tile_pool(name="sbz", bufs=6))
        sbg = ctx.enter_context(tc.tile_pool(name="sbg", bufs=3))
        sbt = ctx.enter_context(tc.tile_pool(name="sbt", bufs=3))
        sbm = ctx.enter_context(tc.tile_pool(name="sbm", bufs=3))
        sboh = ctx.enter_context(tc.tile_pool(name="sboh", bufs=4))
        psA = ctx.enter_context(tc.tile_pool(name="psA", bufs=2, space="PSUM"))
        psB = ctx.enter_context(tc.tile_pool(name="psB", bufs=2, space="PSUM"))
        psS = ctx.enter_context(tc.tile_pool(name="psS", bufs=2, space="PSUM"))
        psT = ctx.enter_context(tc.tile_pool(name="psT", bufs=2, space="PSUM"))

        dcolS = cst.tile([P, NB], f32)
        nc.sync.dma_start(out=dcolS[:], in_=dcold[:, :])
        tns = cst.tile([P, TILES], i32)
        nc.sync.dma_start(out=tns[:], in_=tnd[:, :])
        l2S = cst.tile([2, P], bf16)
        nc.sync.dma_start(out=l2S[:], in_=l2d[:, :])
        idenS = cst.tile([P, P], bf16)
        nc.sync.dma_start(out=idenS[:], in_=idend[:, :])
        hwS = cst.tile([HC, 1], bf16)
        nc.sync.dma_start(out=hwS[:], in_=hwd[:, :])
        WextS = cst.tile([IN_DIM, NLAYERS * DZ], bf16)
        for li in range(NLAYERS):
            nc.sync.dma_start(out=WextS[:, li * DZ:(li + 1) * DZ], in_=Wext[li, :, :])
        iota_i = cst.tile([P, P], i32)
        nc.gpsimd.iota(iota_i[:], pattern=[[1, P]], base=0, channel_multiplier=0)
        iotaR = cst.tile([P, P], bf16)
        nc.vector.tensor_copy(out=iotaR[:], in_=iota_i[:])
        logitS = cst.tile([1, VSH], f32)

        for _rep in range(repeat):
         for li in range(NLAYERS):
            WextL = WextS[:, li * DZ:(li + 1) * DZ]
            # ---------- Phase 1 ----------
            WCH = 4 * P
            for wc in range(Vp // WCH):
                if li == 0:
                    hT = sb1.tile([IN_DIM, WCH], bf16, tag="hT")
                    nc.sync.dma_start(out=hT[:], in_=xT[:, wc * WCH:(wc + 1) * WCH])
                else:
                    hT = sb1.tile([IN_DIM, WCH], bf16, tag="hT")
                    n0 = wc * WCH
                    done = 0
                    while done < WCH:
                        g0 = n0 + done
                        blkc, colc = g0 // VSH, g0 % VSH
                        w = min(WCH - done, VSH - colc)
                        nc.sync.dma_start(out=hT[:, done:done + w],
                                          in_=hfull.ap()[blkc, :, colc:colc + w])
                        done += w
                zs = sbzs.tile([P, 4, DZU], u16, tag="zs")
                nc.gpsimd.memset(zs[:, :, HC + 16:DZU], 0)
                for sub in range(4):
                    zc = psB.tile([P, GROUP], f32, space="PSUM", tag="big")
                    nc.tensor.matmul(out=zc[:, 0:DZ], lhsT=hT[:, sub * P:(sub + 1) * P],
                                     rhs=WextL, start=True, stop=True)
                    if sub % 2 == 0:
                        nc.scalar.activation(out=zs[:, sub, 0:HC].bitcast(bf16),
                                             in_=zc[:, 0:HC], func=AF.Copy)
                    else:
                        nc.vector.tensor_copy(out=zs[:, sub, 0:HC].bitcast(bf16),
                                              in_=zc[:, 0:HC])
                    nc.vector.tensor_copy(out=zs[:, sub, HC:HC + 16].bitcast(f32),
                                          in_=zc[:, HC:DZ])
                nc.scalar.dma_start(
                    out=ztab.ap()[wc * WCH:(wc + 1) * WCH, :].rearrange(
                        "(s p) c -> p s c", s=4, p=P),
                    in_=zs[:])
            # ---------- Phase 2 ----------
            for k in range(TILES):
                gk = g_per_slot[k]
                rhs2 = sbr2.tile([2, GMAX * GROUP], bf16, tag="rhs2")
                nc.sync.dma_start(
                    out=rhs2[:, 0:gk * GROUP],
                    in_=drow2d[:, grp0[k] * GROUP:(grp0[k] + gk) * GROUP])
                idT = sbr2.tile([P, GMAX * IPC], i16, tag="idT")
                nc.sync.dma_start(
                    out=idT[:, 0:gk * IPC],
                    in_=idxd[:, grp0[k] * IPC:(grp0[k] + gk) * IPC])
                sdr = sbsd.tile([P, 8], u16, tag="sdr")
                nc.gpsimd.indirect_dma_start(
                    out=sdr[:], out_offset=None, in_=ztab.ap()[:, :],
                    in_offset=IndirectOffsetOnAxis(ap=tns[:, k:k + 1], axis=0),
                    element_offset=HC + 8)
                sdb = sbsd.tile([P, 4], bf16, tag="sdb")
                nc.vector.tensor_copy(out=sdb[:], in_=sdr[:].bitcast(f32))
                aggden = psA.tile([P, HC + 4], f32, space="PSUM", tag="aggden")
                for g in range(gk):
                    col0 = blk0[k] + g * BPG
                    ge = sbz.tile([P, BPG, DZU], u16, tag="ge")
                    inv = ztab.ap()[0:HALF, :] if g < gA[k] else ztab.ap()[HALF:Vp, :]
                    nc.gpsimd.dma_gather(
                        out_ap=ge[:], in_ap=inv,
                        idxs_ap=idT[:, g * IPC:(g + 1) * IPC],
                        num_idxs=GROUP, num_idxs_reg=GROUP, elem_size=DZU)
                    dbc2 = psB.tile([P, GROUP], f32, space="PSUM", tag="big")
                    nc.tensor.matmul(out=dbc2[:], lhsT=l2S[:],
                                     rhs=rhs2[:, g * GROUP:(g + 1) * GROUP],
                                     start=True, stop=True)
                    ohTa = sbg.tile([P, GROUP], f32, tag="ohTa")
                    nc.scalar.activation(out=ohTa[:], in_=dbc2[:], func=AF.Abs)
                    ohT = sbg.tile([P, GROUP], bf16, tag="ohT")
                    nc.scalar.activation(out=ohT[:], in_=ohTa[:], func=AF.Relu,
                                         scale=-1.0, bias=1.0)
                    sdeP = psS.tile([P, BPG, 4], f32, space="PSUM", tag="sde")
                    for b in range(BPG):
                        nc.tensor.matmul(out=sdeP[:, b, :],
                                         lhsT=ohT[:, b * P:(b + 1) * P],
                                         rhs=sdb[:], start=True, stop=True)
                    escS = sbt.tile([P, BPG, 4], f32, tag="escS")
                    nc.vector.tensor_tensor(
                        out=escS[:], in0=ge[:, :, HC:HC + 8].bitcast(f32),
                        in1=sdeP[:], op=OP.add)
                    escL = sbt.tile([P, BPG, 4], f32, tag="escL")
                    nc.vector.scalar_tensor_tensor(
                        out=escL[:], in0=escS[:], scalar=NEG, in1=escS[:],
                        op0=OP.mult, op1=OP.max)
                    # msgG cols 0:HC = z*exp, cols HC:HC+4 = exp (fused den)
                    msgG = sbm.tile([P, BPG, HC + 4], bf16, tag="msgG")
                    nc.scalar.activation(out=msgG[:, :, HC:HC + 4], in_=escL[:],
                                         func=AF.Exp)
                    nc.vector.tensor_tensor(
                        out=msgG[:, :, 0:HC].rearrange(
                            "p g (h c) -> p g h c", h=HEADS, c=COUT),
                        in0=ge[:, :, 0:HC].bitcast(bf16).rearrange(
                            "p g (h c) -> p g h c", h=HEADS, c=COUT),
                        in1=msgG[:, :, HC:HC + 4].unsqueeze(-1).broadcast_to(
                            [P, BPG, HEADS, COUT]),
                        op=OP.mult)
                    for b in range(BPG):
                        col = col0 + b
                        first = (g == 0 and b == 0)
                        last = (g == gk - 1 and b == BPG - 1)
                        oh = sboh.tile([P, P], bf16, tag="oh")
                        nc.vector.tensor_scalar(out=oh[:], in0=iotaR[:],
                                                scalar1=dcolS[:, col:col + 1],
                                                scalar2=None, op0=OP.is_equal)
                        nc.tensor.matmul(out=aggden[:, 0:HC + 4], lhsT=oh[:],
                                         rhs=msgG[:, b, :], start=first, stop=last,
                                         skip_group_check=True)
                # ---- finalize tile k ----
                dsb = sbt.tile([P, 4], f32, tag="dsb")
                nc.vector.tensor_scalar(out=dsb[:], in0=aggden[:, HC:HC + 4],
                                        scalar1=1e-9, scalar2=None, op0=OP.add)
                rec = sbt.tile([P, 4], f32, tag="rec")
                nc.vector.reciprocal(out=rec[:], in_=dsb[:])
                xn = sbt.tile([P, HC], f32, tag="xn")
                nc.vector.tensor_tensor(
                    out=xn[:].rearrange("p (h c) -> p h c", h=HEADS, c=COUT),
                    in0=aggden[:, 0:HC].rearrange("p (h c) -> p h c", h=HEADS, c=COUT),
                    in1=rec[:].unsqueeze(-1).broadcast_to([P, HEADS, COUT]),
                    op=OP.mult)
                tmin = sbt.tile([P, HC], f32, tag="tmin")
                nc.vector.tensor_scalar(out=tmin[:], in0=xn[:], scalar1=0.0,
                                        scalar2=None, op0=OP.min)
                texp = sbt.tile([P, HC], f32, tag="texp")
                nc.scalar.activation(out=texp[:], in_=tmin[:], func=AF.Exp)
                hsum = sbt.tile([P, HC], f32, tag="hsum")
                nc.vector.scalar_tensor_tensor(
                    out=hsum[:], in0=xn[:], scalar=0.0, in1=texp[:],
                    op0=OP.max, op1=OP.add)
                hb16 = sbt.tile([P, HC], bf16, tag="hb16")
                nc.vector.tensor_scalar(out=hb16[:], in0=hsum[:], scalar1=-1.0,
                                        scalar2=None, op0=OP.add)
                pst = psT.tile([P, P], f32, space="PSUM", tag="pst")
                nc.tensor.matmul(out=pst[:], lhsT=hb16[:], rhs=idenS[:],
                                 start=True, stop=True)
                if li < NLAYERS - 1:
                    hT16 = sbt.tile([P, P], bf16, tag="hT16")
                    nc.scalar.activation(out=hT16[:], in_=pst[:], func=AF.Copy)
                    nc.scalar.dma_start(out=hsh.ap()[:, k * P:(k + 1) * P], in_=hT16[:])
                else:
                    h3T = sbt.tile([P, P], bf16, tag="hT16")
                    nc.scalar.activation(out=h3T[:], in_=pst[:], func=AF.Copy)
                    lgt = psT.tile([P, P], f32, space="PSUM", tag="pst")
                    lg = lgt[0:1, :]
                    nc.tensor.matmul(out=lg, lhsT=hwS[:], rhs=h3T[:], start=True,
                                     stop=True)
                    nc.vector.tensor_scalar(out=logitS[:, k * P:(k + 1) * P], in0=lg,
                                            scalar1=hb, scalar2=None, op0=OP.add)
            if li < NLAYERS - 1:
                nc.gpsimd.collective_compute(
                    "AllGather", mybir.AluOpType.bypass,
                    replica_groups=[list(range(NC))],
                    ins=[hsh.ap()[:, :]], outs=[hfull.ap()[:, :, :]])
        nc.sync.dma_start(out=out[None, :], in_=logitS[:])
    nc.compile()
    return nc


def gat_reference_np(x, edge_index, Ws, a_src, a_dst, head_w, head_b):
    """Numpy reference (same math as reference.py) for small-scale validation."""
    V = x.shape[0]
    src = np.asarray(edge_index[0]); dst = np.asarray(edge_index[1])
    h = np.asarray(x, np.float64)
    for li in range(len(Ws)):
        z = (h @ np.asarray(Ws[li], np.float64).T).reshape(V, HEADS, COUT)
        ss = np.einsum("vhc,hc->vh", z, np.asarray(a_src[li], np.float64))
        sd = np.einsum("vhc,hc->vh", z, np.asarray(a_dst[li], np.float64))
        e = ss[src] + sd[dst]
        e = np.where(e > 0, e, NEG * e)
        m = np.full((V, HEADS), -np.inf); np.maximum.at(m, dst, e)
        m = np.maximum(m, -1e9)
        ex = np.exp(e - m[dst])
        den = np.zeros((V, HEADS)); np.add.at(den, dst, ex)
        alpha = ex / (den[dst] + 1e-9)
        msg = z[src] * alpha[:, :, None]
        agg = np.zeros((V, HEADS, COUT)); np.add.at(agg, dst, msg)
        h = np.where(agg > 0, agg, np.expm1(agg)).reshape(V, HC)
    return (h @ np.asarray(head_w, np.float64).T + np.asarray(head_b)).reshape(V)


# ======================= runner =======================

import time
import jax
from jax.sharding import Mesh, PartitionSpec
from jax.experimental.shard_map import shard_map

from concourse import bass2jax
from concourse.bass2jax import _bass_exec_p, install_neuronx_cc_hook, partition_id_tensor


class SpmdRunner:
    def __init__(self, nc, n_cores: int):
        install_neuronx_cc_hook()
        assert nc.dbg_addr is None or not nc.dbg_callbacks
        self.nc = nc
        self.n_cores = n_cores
        partition_name = nc.partition_id_tensor.name if nc.partition_id_tensor else None

        in_names, out_names, out_avals, zero_outs = [], [], [], []
        for alloc in nc.m.functions[0].allocations:
            if not isinstance(alloc, mybir.MemoryLocationSet):
                continue
            name = alloc.memorylocations[0].name
            if alloc.kind == "ExternalInput":
                if name != partition_name and name != (nc.dbg_addr.name if nc.dbg_addr else None):
                    in_names.append(name)
            elif alloc.kind == "ExternalOutput":
                out_names.append(name)
                shape = tuple(alloc.tensor_shape)
                dtype = mybir.dt.np(alloc.dtype)
                out_avals.append(jax.core.ShapedArray(shape, dtype))
                zero_outs.append(np.zeros(shape, dtype))
        self.in_names, self.out_names = in_names, out_names
        self.out_avals, self.zero_outs = out_avals, zero_outs
        n_params = len(in_names)
        self.n_params = n_params
        n_outs = len(out_avals)

        all_in_names = list(in_names) + list(out_names)
        if nc.dbg_addr is not None:
            all_in_names.append(nc.dbg_addr.name)
        if partition_name is not None:
            all_in_names.append(partition_name)

        dbg_name = nc.dbg_addr.name if nc.dbg_addr is not None else None

        def _body(*args):
            operands = list(args)
            if dbg_name is not None:
                operands.append(np.zeros((1, 2), np.uint32))
            if partition_name is not None:
                operands.append(partition_id_tensor())
            outs = _bass_exec_p.bind(
                *operands,
                out_avals=tuple(out_avals),
                in_names=tuple(all_in_names),
                out_names=tuple(out_names),
                lowering_input_output_aliases=(),
                sim_require_finite=True,
                sim_require_nnan=True,
                nc=nc,
            )
            return tuple(outs)

        devices = jax.devices()[:n_cores]
        assert len(devices) == n_cores
        self.mesh = Mesh(np.asarray(devices), ("core",))
        in_specs = (PartitionSpec("core"),) * (n_params + n_outs)
        out_specs = (PartitionSpec("core"),) * n_outs
        self.donate = tuple(range(n_params, n_params + n_outs))
        self.fn = jax.jit(
            shard_map(_body, mesh=self.mesh, in_specs=in_specs,
                      out_specs=out_specs, check_rep=False),
            donate_argnums=self.donate, keep_unused=True,
        )
        self.concat_in = None

    def load_inputs(self, in_maps):
        """Concat per-core inputs and push to devices once."""
        assert len(in_maps) == self.n_cores
        per_core = [[np.asarray(m[name]) for name in self.in_names] for m in in_maps]
        concat = [np.concatenate([per_core[c][i] for c in range(self.n_cores)], axis=0)
                  for i in range(self.n_params)]
        sh = jax.sharding.NamedSharding(self.mesh, PartitionSpec("core"))
        self.concat_in = [jax.device_put(a, sh) for a in concat]

    def _zeros(self):
        sh = jax.sharding.NamedSharding(self.mesh, PartitionSpec("core"))
        return [jax.device_put(np.zeros((self.n_cores * z.shape[0], *z.shape[1:]), z.dtype), sh)
                for z in self.zero_outs]

    def run(self):
        outs = self.fn(*self.concat_in, *self._zeros())
        jax.block_until_ready(outs)
        return [
            {name: np.asarray(outs[i]).reshape(self.n_cores, *self.out_avals[i].shape)[c]
             for i, name in enumerate(self.out_names)}
            for c in range(self.n_cores)
        ]

    def time(self, iters=8, warmup=2):
        """Per-call wall time (s) for the jitted executable, zeros pre-staged."""
        zs = [self._zeros() for _ in range(iters + warmup)]
        for i in range(warmup):
            jax.block_until_ready(self.fn(*self.concat_in, *zs[i]))
        ts = []
        for i in range(iters):
            t0 = time.perf_counter()
            jax.block_until_ready(self.fn(*self.concat_in, *zs[warmup + i]))
            ts.append(time.perf_counter() - t0)
        return min(ts), ts


# ======================= driver (self-contained kernel) =======================
import jax as _jax

_CACHE = {}
LAST_EXEC_NS = None


def _floor_nc(ncores):
    """Tiny kernel to estimate the per-call dispatch floor."""
    nc = bacc.Bacc("TRN2", target_bir_lowering=False, debug=False, num_devices=ncores)
    a = nc.dram_tensor("a", [P, 64], mybir.dt.float32, kind="ExternalInput").ap()
    b = nc.dram_tensor("b", [P, 64], mybir.dt.float32, kind="ExternalOutput").ap()
    with tile.TileContext(nc) as tc, ExitStack() as ctx:
        sb = ctx.enter_context(tc.tile_pool(name="sb", bufs=2))
        t = sb.tile([P, 64], mybir.dt.float32)
        nc.sync.dma_start(out=t[:], in_=a[:, :])
        nc.sync.dma_start(out=b[:, :], in_=t[:])
    nc.compile()
    return nc


def kernel(x, edge_index, Ws, a_src, a_dst, head_w, head_b):
    NC = 8
    V = int(np.asarray(x).shape[0])
    cfg = make_cfg(V, NC, tiles_per_core=(V + NC * P - 1) // (NC * P))
    in_maps, meta = host_prep(cfg, x, edge_index, Ws, a_src, a_dst, head_w, head_b)
    key = (V, tuple(meta["g_per_slot"]), tuple(meta["b_per_slot"]))
    if key not in _CACHE:
        nc = build_nc(cfg, meta)
        r = SpmdRunner(nc, NC)
        _CACHE[key] = r
    r = _CACHE[key]
    r.load_inputs(in_maps)
    res = r.run()
    out = np.concatenate([res[c]["out"] for c in range(NC)])[:V]
    return out.astype(np.float32)


def measure(iters=16):
    """Estimate HW exec ns via interleaved kernel/floor timing (drift-robust)."""
    import time as _time
    global LAST_EXEC_NS
    assert _CACHE, "call kernel() first"
    r = next(iter(_CACHE.values()))
    fnc = _floor_nc(r.n_cores)
    fr = SpmdRunner(fnc, r.n_cores)
    fr.load_inputs([{"a": np.zeros((P, 64), np.float32)}] * r.n_cores)
    fr.run()
    r.run()
    diffs, ks, fs = [], [], []
    for _ in range(iters):
        z = r._zeros()
        t0 = _time.perf_counter()
        _jax.block_until_ready(r.fn(*r.concat_in, *z))
        tk = _time.perf_counter() - t0
        zf = fr._zeros()
        t0 = _time.perf_counter()
        _jax.block_until_ready(fr.fn(*fr.concat_in, *zf))
        tf = _time.perf_counter() - t0
        ks.append(tk); fs.append(tf); diffs.append(tk - tf)
    diffs.sort()
    med = diffs[len(diffs) // 2]
    LAST_EXEC_NS = int(max(0.0, med) * 1e9)
    return LAST_EXEC_NS, sorted(ks)[len(ks)//2], sorted(fs)[len(fs)//2]
